# revision 1
# baseline (speedup 1.0000x reference)
"""Trainium2 Bass kernel for nn_AttentionToken.

reference semantics (per full input (S=512, B=2048, E=30)):
    squish  = tanh(x @ W + bias[:,0])          # (S,B,E)
    attn    = tanh(squish @ proj[:,0])         # (S,B)
    attn_n  = softmax over S, per batch        # (B,S)
    out     = stack([x.T(b,s,e), x.T * attn_n[:, :, None]], axis=1)  # (B,2,S,E)

Sharding: data-parallel over batch, 8 cores x 256 batches.

Per-core layout: batch on partitions (2 groups of 128), (s, e) on the free
dim, x resident in SBUF between the attention pass and the scaled-output
pass.  The 30x30 matmul is done as a block-diagonal (W x 4) 120x120 matmul
over PE-transposed (120, 128) panels covering 4 sequence positions x 128
batches per chunk.
"""

from contextlib import ExitStack

import numpy as np

import concourse.bass as bass
import concourse.tile as tile
from concourse import mybir
from concourse.bass_utils import run_bass_kernel_spmd
from concourse.masks import make_identity
from concourse.vector_clock import ScopedClock


class _TileContextSplitDrain(tile.TileContext):
    """TileContext whose exit drain stays within the 1-sem-wait-per-
    instruction encoding limit of this walrus build.

    The stock ``_drain_and_barrier`` attaches the whole global clock to a
    single Drain, which codegen rejects ("Too many sync wait commands").
    Emit one standalone SP wait per semaphore instead, then a clean drain.
    """

    def _drain_and_barrier(self, tick_clock, wait_clock):
        nc = self.nc
        with nc.discard():
            probe = nc.sync.drain()
            wait_clock.add_sem_waits(
                probe.ins, ScopedClock({None: tick_clock.global_clock})
            )
            si = probe.ins.sync_info
            waits = list(si.on_wait) if si and si.on_wait else []
        assert self.sems is not None
        alloc = self.sems.allocated()
        by_num = {h.num: h for h in alloc.values()}
        for w in waits:
            h = by_num.get(w.id)
            assert h is not None, (w.id, w.ant_name, sorted(by_num))
            nc.sync.wait_ge(h, w.wait_value)
        nc.sync.drain()
        nc.all_engine_barrier()
        popped = nc._tile_sem_poison_stack.pop()
        assert popped is self._sem_poison
        nc.clear_and_free_semaphores(list(alloc.values()))
        nc.all_engine_barrier()

S = 512
B = 2048
E = 30
N_CORES = 8
BC = B // N_CORES          # batches per core (256)
PG = 128                   # batches per group (partition dim)
N_GROUPS = BC // PG        # 2
SCHUNK = 4                 # sequence positions per PE chunk (4*30 = 120 <= 128)
N_CHUNKS = S // SCHUNK     # 128
KB = SCHUNK * E            # 120: block-diag contraction/output size
F32 = mybir.dt.float32


def _split_multi_waits(nc, max_waits=1):
    """This walrus build encodes at most one sem-wait per instruction; the
    Tile scheduler emits up to ~3.  Hoist extra waits onto standalone
    EventSemaphore instructions on the same engine, just before the owner.
    """
    n = 0
    for f in nc.m.functions:
        for bb in f.blocks:
            out = []
            for ins in bb.instructions:
                si = ins.sync_info
                waits = list(si.on_wait) if si and si.on_wait else []
                if len(waits) > max_waits:
                    for w in waits[:-max_waits]:
                        ev = mybir.InstEventSemaphore(
                            name=f"wsplit-{n}",
                            opcode="EventSemaphore",
                            engine=ins.engine,
                            sync_info=mybir.SyncInfo(on_wait=[w], on_update=[]),
                        )
                        n += 1
                        out.append(ev)
                    ins.sync_info = mybir.SyncInfo(
                        on_wait=waits[-max_waits:],
                        on_update=list(si.on_update or []),
                    )
                out.append(ins)
            bb.instructions = out


def _build_program():
    nc = bass.Bass()
    x_d = nc.declare_dram_parameter("input", [S, BC, E], F32, isOutput=False)
    w4_d = nc.declare_dram_parameter("W4", [KB, KB], F32, isOutput=False)
    b4_d = nc.declare_dram_parameter("bias4", [KB, 1], F32, isOutput=False)
    p4_d = nc.declare_dram_parameter("proj4", [KB, SCHUNK], F32, isOutput=False)
    out_d = nc.declare_dram_parameter("output", [BC, 2, S, E], F32, isOutput=True)

    with _TileContextSplitDrain(nc) as tc, ExitStack() as ctx:
        consts = ctx.enter_context(tc.tile_pool(name="consts", bufs=1))
        xpool = ctx.enter_context(tc.tile_pool(name="x", bufs=2))
        xspool = ctx.enter_context(tc.tile_pool(name="xs", bufs=3))
        xt_pool = ctx.enter_context(tc.tile_pool(name="xt", bufs=3))
        sq_pool = ctx.enter_context(tc.tile_pool(name="sq", bufs=3))
        sm_pool = ctx.enter_context(tc.tile_pool(name="sm", bufs=2))
        ps_tp = ctx.enter_context(tc.tile_pool(name="ps_tp", bufs=2, space="PSUM"))
        ps_xt = ctx.enter_context(tc.tile_pool(name="ps_xt", bufs=2, space="PSUM"))
        ps_sq = ctx.enter_context(tc.tile_pool(name="ps_sq", bufs=2, space="PSUM"))
        ps_at = ctx.enter_context(tc.tile_pool(name="ps_at", bufs=2, space="PSUM"))

        ident = consts.tile([128, 128], F32)
        make_identity(nc, ident[:])
        w4_sb = consts.tile([KB, KB], F32)
        nc.sync.dma_start(out=w4_sb[:], in_=w4_d[:, :])
        b4_sb = consts.tile([KB, 1], F32)
        nc.sync.dma_start(out=b4_sb[:], in_=b4_d[:, :])
        p4_sb = consts.tile([KB, SCHUNK], F32)
        nc.sync.dma_start(out=p4_sb[:], in_=p4_d[:, :])

        SB = S // 4  # 128: s-block size

        for g in range(N_GROUPS):
            b0 = g * PG
            xg = xpool.tile([PG, S, E], F32)
            attn_ps = ps_at.tile([PG, S], F32)
            for j in range(4):
                s0 = j * SB
                # line-rate load: s on partitions, (b, e) contiguous 15.4KB/run
                xs = xspool.tile([SB, PG, E], F32)
                nc.sync.dma_start(
                    out=xs[:], in_=x_d[s0 : s0 + SB, b0 : b0 + PG, :]
                )
                # PE-transpose (s, b) -> (b, s) one e-slice at a time
                for e in range(E):
                    tp = ps_tp.tile([PG, SB], F32)
                    nc.tensor.transpose(tp[:], xs[:, :, e], ident[:])
                    nc.vector.tensor_copy(xg[:, s0 : s0 + SB, e], tp[:])
                # unscaled half of the output: independent of attn
                nc.sync.dma_start(
                    out=out_d[b0 : b0 + PG, 0, s0 : s0 + SB, :],
                    in_=xg[:, s0 : s0 + SB, :],
                )
                # attention chunks for this s-block (4 seq positions each)
                for c in range(SB // SCHUNK):
                    sc = s0 + c * SCHUNK
                    chunk = xg[:, sc : sc + SCHUNK, :]  # (128, 4, 30)
                    xt_ps = ps_xt.tile([KB, PG], F32)
                    nc.tensor.transpose(xt_ps[:], chunk, ident[:])
                    xt_sb = xt_pool.tile([KB, PG], F32)
                    nc.vector.tensor_copy(xt_sb[:], xt_ps[:])
                    sq_ps = ps_sq.tile([KB, PG], F32)
                    nc.tensor.matmul(
                        sq_ps[:], w4_sb[:], xt_sb[:], start=True, stop=True
                    )
                    sq_sb = sq_pool.tile([KB, PG], F32)
                    nc.scalar.activation(
                        sq_sb[:], sq_ps[:], mybir.ActivationFunctionType.Tanh,
                        bias=b4_sb[:, 0:1], scale=1.0,
                    )
                    nc.tensor.matmul(
                        attn_ps[:, sc : sc + SCHUNK], sq_sb[:], p4_sb[:],
                        start=True, stop=True,
                    )

            # attn = tanh(attn_pre); softmax over s (free axis) per batch
            attn_sb = sm_pool.tile([PG, S], F32)
            nc.scalar.activation(
                attn_sb[:], attn_ps[:], mybir.ActivationFunctionType.Tanh
            )
            mx = sm_pool.tile([PG, 1], F32)
            nc.vector.reduce_max(out=mx[:], in_=attn_sb[:], axis=mybir.AxisListType.X)
            negmx = sm_pool.tile([PG, 1], F32)
            nc.vector.tensor_scalar_mul(negmx[:], mx[:], -1.0)
            p_sb = sm_pool.tile([PG, S], F32)
            ssum = sm_pool.tile([PG, 1], F32)
            nc.scalar.activation(
                p_sb[:], attn_sb[:], mybir.ActivationFunctionType.Exp,
                bias=negmx[:, 0:1], scale=1.0, accum_out=ssum[:, 0:1],
            )
            rcp = sm_pool.tile([PG, 1], F32)
            nc.vector.reciprocal(rcp[:], ssum[:])
            nc.vector.tensor_scalar_mul(p_sb[:], p_sb[:], rcp[:, 0:1])

            # scaled half of the output: in-place scale of xg (out0 for the
            # block has already been stored), then line-rate store
            for j in range(4):
                s0 = j * SB
                pslice = p_sb[:, s0 : s0 + SB]
                pb = bass.AP(
                    tensor=pslice.tensor,
                    offset=pslice.offset,
                    ap=list(pslice.ap) + [[0, E]],
                )
                nc.vector.tensor_tensor(
                    out=xg[:, s0 : s0 + SB, :], in0=xg[:, s0 : s0 + SB, :],
                    in1=pb, op=mybir.AluOpType.mult,
                )
                nc.sync.dma_start(
                    out=out_d[b0 : b0 + PG, 1, s0 : s0 + SB, :],
                    in_=xg[:, s0 : s0 + SB, :],
                )
    _split_multi_waits(nc)
    return nc


_NC_CACHE = None


def _get_program():
    global _NC_CACHE
    if _NC_CACHE is None:
        _NC_CACHE = _build_program()
    return _NC_CACHE


def kernel(input, W, bias, proj, _want_trace=False, _trace_dir=None):
    x = np.ascontiguousarray(np.asarray(input, dtype=np.float32))
    W = np.asarray(W, dtype=np.float32)
    bias = np.asarray(bias, dtype=np.float32)
    proj = np.asarray(proj, dtype=np.float32)
    assert x.shape == (S, B, E)

    w4 = np.zeros((KB, KB), np.float32)
    b4 = np.zeros((KB, 1), np.float32)
    p4 = np.zeros((KB, SCHUNK), np.float32)
    for g in range(SCHUNK):
        w4[g * E : (g + 1) * E, g * E : (g + 1) * E] = W
        b4[g * E : (g + 1) * E, 0] = bias[:, 0]
        p4[g * E : (g + 1) * E, g] = proj[:, 0]

    nc = _get_program()
    in_maps = []
    for c in range(N_CORES):
        shard = np.ascontiguousarray(x[:, c * BC : (c + 1) * BC, :])
        in_maps.append({"input": shard, "W4": w4, "bias4": b4, "proj4": p4})

    res = run_bass_kernel_spmd(
        nc, in_maps, list(range(N_CORES)), trace=_want_trace, tmpdir=_trace_dir
    )
    out = np.concatenate([res.results[c]["output"] for c in range(N_CORES)], axis=0)
    if _want_trace:
        return out, res
    return out



# revision 14
# speedup vs baseline: 1.1938x; 1.1938x over previous
"""Trainium2 Bass kernel for nn_AttentionToken.

reference semantics (per full input (S=512, B=2048, E=30)):
    squish  = tanh(x @ W + bias[:,0])          # (S,B,E)
    attn    = tanh(squish @ proj[:,0])         # (S,B)
    attn_n  = softmax over S, per batch        # (B,S)
    out     = stack([x.T(b,s,e), x.T * attn_n[:, :, None]], axis=1)  # (B,2,S,E)

Sharding: data-parallel over batch, 8 cores x 256 batches.

Per-core design (v2, bf16 internal pipeline):
  - x loaded s-major (s on partitions, (b,e) free) at DMA line rate, converted
    to bf16 once on the Scalar engine.
  - b-major fp32 copy (xg) built with bf16 PE transposes (1 cycle/row) whose
    PSUM results are drained in batched, up-converting GpSimd copies; out0
    stores stream from xg at line rate.
  - matmul path: 128x120 bf16 PE transposes pack 4 batches x 30 embed onto
    120 partitions; the 30x30 GEMM runs as a block-diagonal 120x120 bf16
    matmul with a 512-wide moving operand (1 cycle/row).  tanh on Scalar.
  - the tiny proj contraction keeps proj as the stationary operand (4-column
    weight load) with the 512-wide squish tile moving.
  - batches are packed into quads so attention lands as (32, (cgrp, s)) tiles
    that two small PE transpose rounds turn into natural (b, s) layout.
  - softmax over the free axis, in-place scale of xg, line-rate out1 stores.
"""

from contextlib import ExitStack

import numpy as np

import concourse.bass as bass
import concourse.tile as tile
from concourse import mybir
from concourse.bass_utils import run_bass_kernel_spmd
from concourse.masks import make_identity
from concourse.vector_clock import ScopedClock


class _TileContextSplitDrain(tile.TileContext):
    """TileContext whose exit drain stays within the 1-sem-wait-per-
    instruction encoding limit of this walrus build.

    The stock ``_drain_and_barrier`` attaches the whole global clock to a
    single Drain, which codegen rejects ("Too many sync wait commands").
    Emit one standalone SP wait per semaphore instead, then a clean drain.
    """

    def _drain_and_barrier(self, tick_clock, wait_clock):
        nc = self.nc
        with nc.discard():
            probe = nc.sync.drain()
            wait_clock.add_sem_waits(
                probe.ins, ScopedClock({None: tick_clock.global_clock})
            )
            si = probe.ins.sync_info
            waits = list(si.on_wait) if si and si.on_wait else []
        assert self.sems is not None
        alloc = self.sems.allocated()
        by_num = {h.num: h for h in alloc.values()}
        for w in waits:
            h = by_num.get(w.id)
            assert h is not None, (w.id, w.ant_name, sorted(by_num))
            nc.sync.wait_ge(h, w.wait_value)
        nc.sync.drain()
        nc.all_engine_barrier()
        popped = nc._tile_sem_poison_stack.pop()
        assert popped is self._sem_poison
        nc.clear_and_free_semaphores(list(alloc.values()))
        nc.all_engine_barrier()

S = 512
B = 2048
E = 30
N_CORES = 8
BC = B // N_CORES          # batches per core (256)
SB = 128                   # s-block size (s on partitions)
KB = 4 * E                 # 120: block-diag contraction/output size
F32 = mybir.dt.float32
BF16 = mybir.dt.bfloat16
AF = mybir.ActivationFunctionType


def _split_multi_waits(nc, max_waits=1):
    """This walrus build encodes at most one sem-wait per instruction; the
    Tile scheduler emits up to ~3.  Hoist extra waits onto standalone
    EventSemaphore instructions on the same engine, just before the owner.
    """
    n = 0
    for f in nc.m.functions:
        for bb in f.blocks:
            out = []
            for ins in bb.instructions:
                si = ins.sync_info
                waits = list(si.on_wait) if si and si.on_wait else []
                if len(waits) > max_waits:
                    for w in waits[:-max_waits]:
                        ev = mybir.InstEventSemaphore(
                            name=f"wsplit-{n}",
                            opcode="EventSemaphore",
                            engine=ins.engine,
                            sync_info=mybir.SyncInfo(on_wait=[w], on_update=[]),
                        )
                        n += 1
                        out.append(ev)
                    ins.sync_info = mybir.SyncInfo(
                        on_wait=waits[-max_waits:],
                        on_update=list(si.on_update or []),
                    )
                out.append(ins)
            bb.instructions = out


def _reorder_free(ap, perm):
    """Return the same AP with its free dims permuted (partition dim kept)."""
    dims = list(ap.ap)
    free = dims[1:]
    return bass.AP(
        tensor=ap.tensor,
        offset=ap.offset,
        ap=[dims[0]] + [free[i] for i in perm],
    )


def _build_program():
    nc = bass.Bass()
    x_d = nc.declare_dram_parameter("input", [S, BC, E], F32, isOutput=False)
    w4_d = nc.declare_dram_parameter("W4", [KB, KB], BF16, isOutput=False)
    b4_d = nc.declare_dram_parameter("bias4", [KB, 1], F32, isOutput=False)
    p4_d = nc.declare_dram_parameter("proj4", [KB, 256], BF16, isOutput=False)
    out_d = nc.declare_dram_parameter("output", [BC, 2, S, E], F32, isOutput=True)

    # e-slice transpose batches: (start, count) draining into one PSUM tile
    EBATCH = [(0, 8), (8, 8), (16, 8), (24, 6)]

    with _TileContextSplitDrain(nc) as tc, ExitStack() as ctx:
        consts = ctx.enter_context(tc.tile_pool(name="consts", bufs=1))
        xs_pool = ctx.enter_context(tc.tile_pool(name="xs", bufs=2))
        x16_pool = ctx.enter_context(tc.tile_pool(name="x16", bufs=2))
        xg_pool = ctx.enter_context(tc.tile_pool(name="xg", bufs=1))
        xt_pool = ctx.enter_context(tc.tile_pool(name="xt", bufs=3))
        sq_pool = ctx.enter_context(tc.tile_pool(name="sq", bufs=4))
        at_pool = ctx.enter_context(tc.tile_pool(name="at", bufs=6))
        atS_pool = ctx.enter_context(tc.tile_pool(name="atS", bufs=2))
        atT_pool = ctx.enter_context(tc.tile_pool(name="atT", bufs=2))
        sm_pool = ctx.enter_context(tc.tile_pool(name="sm", bufs=2))
        smv_pool = ctx.enter_context(tc.tile_pool(name="smv", bufs=8))
        ps_tp = ctx.enter_context(tc.tile_pool(name="ps_tp", bufs=2, space="PSUM"))
        ps_xt = ctx.enter_context(tc.tile_pool(name="ps_xt", bufs=2, space="PSUM"))
        ps_sq = ctx.enter_context(tc.tile_pool(name="ps_sq", bufs=2, space="PSUM"))
        ps_sm = ctx.enter_context(tc.tile_pool(name="ps_sm", bufs=2, space="PSUM"))

        ident16 = consts.tile([128, 128], BF16)
        make_identity(nc, ident16[:])
        identf = consts.tile([128, 128], F32)
        make_identity(nc, identf[:])
        w4_sb = consts.tile([KB, KB], BF16)
        nc.sync.dma_start(out=w4_sb[:], in_=w4_d[:, :])
        b4_sb = consts.tile([KB, 1], F32)
        nc.sync.dma_start(out=b4_sb[:], in_=b4_d[:, :])
        p4_sb = consts.tile([KB, 256], BF16)
        nc.sync.dma_start(out=p4_sb[:], in_=p4_d[:, :])

        xg = [
            xg_pool.tile([128, S, E], F32, name=f"xg{i}", tag=f"xg{i}")
            for i in range(2)
        ]

        for h in range(2):
            b0 = h * 128
            attn_sbs = []
            for j in range(4):
                s0 = j * SB
                # line-rate load: s on partitions, (b, e) contiguous 15.4KB/run
                xs = xs_pool.tile([SB, 128, E], F32)
                nc.sync.dma_start(
                    out=xs[:], in_=x_d[s0 : s0 + SB, b0 : b0 + 128, :]
                )
                x16 = x16_pool.tile([SB, 128, E], BF16)
                nc.gpsimd.tensor_copy(x16[:], xs[:])

                # b-major conversion: per-e PE transposes (s,b)->(b,s), drained
                # in batched up-converting GpSimd copies into xg
                for bi, (e0, ne) in enumerate(EBATCH):
                    tp = ps_tp.tile([128, 1024], BF16)
                    for k in range(ne):
                        nc.tensor.transpose(
                            tp[:, k * 128 : (k + 1) * 128],
                            x16[:, :, e0 + k],
                            ident16[:],
                        )
                    sl = xg[h][:, s0 : s0 + SB, e0 : e0 + ne]
                    dst = _reorder_free(sl, [1, 0])  # (e outer, s inner)
                    # GpSimd cannot read PSUM: alternate DVE / Scalar drains
                    if bi % 2 == 0:
                        nc.vector.tensor_copy(dst, tp[:, : ne * 128])
                    else:
                        nc.scalar.copy(dst, tp[:, : ne * 128])
                # unscaled half of the output: independent of attn
                nc.sync.dma_start(
                    out=out_d[b0 : b0 + 128, 0, s0 : s0 + SB, :],
                    in_=xg[h][:, s0 : s0 + SB, :],
                )

                # matmul path: quad (m, z, c) holds batches b0 + 32c + 8m + 4z + i
                # so that attention lands at (partition 8m+4z+i, col-group c)
                # with batch-in-half = 32c + partition.
                attn_ps = ps_sm.tile([32, 512], F32, tag="atn")
                for m in range(4):
                    xt_ps = ps_xt.tile([KB, 1024], BF16)
                    for z in range(2):
                        for c in range(4):
                            boff = 32 * c + 8 * m + 4 * z
                            col = (z * 4 + c) * 128
                            nc.tensor.transpose(
                                xt_ps[:, col : col + 128],
                                x16[:, boff : boff + 4, :],
                                ident16[:],
                            )
                    xt_sb = xt_pool.tile([KB, 1024], BF16)
                    nc.vector.tensor_copy(xt_sb[:], xt_ps[:])
                    for z in range(2):
                        sq_ps = ps_sq.tile([KB, 512], F32)
                        nc.tensor.matmul(
                            sq_ps[:], w4_sb[:], xt_sb[:, z * 512 : (z + 1) * 512],
                            start=True, stop=True,
                        )
                        sq_sb = sq_pool.tile([KB, 512], BF16)
                        nc.scalar.activation(
                            sq_sb[:], sq_ps[:], AF.Tanh,
                            bias=b4_sb[:, 0:1], scale=1.0,
                        )
                        # slot's stationary proj matrix only populates output
                        # partitions [4*slot, 4*slot+4); the 8 slot matmuls
                        # accumulate into one (32, 512) PSUM tile.
                        slot = 2 * m + z
                        nc.tensor.matmul(
                            attn_ps[:, :], p4_sb[:, 32 * slot : 32 * slot + 32],
                            sq_sb[:], start=(slot == 0), stop=(slot == 7),
                        )
                attn_sb = at_pool.tile([32, 512], F32)
                nc.scalar.activation(attn_sb[:], attn_ps[:], AF.Tanh)
                attn_sbs.append(attn_sb)

            # bridge: (32 part, (c, s)) attn tiles -> natural (b, s) layout
            attnT = atT_pool.tile([128, S], F32)
            for j in range(4):
                t1 = ps_sm.tile([128, 128], F32, tag="atn")
                for c in range(4):
                    nc.tensor.transpose(
                        t1[:, 32 * c : 32 * c + 32],
                        attn_sbs[j][:, 128 * c : 128 * c + 128],
                        identf[0:32, 0:32],
                    )
                attnS = atS_pool.tile([128, 128], F32)
                nc.vector.tensor_copy(attnS[:], t1[:])
                t2 = ps_sm.tile([128, 128], F32, tag="atn")
                nc.tensor.transpose(t2[:], attnS[:], identf[:])
                nc.vector.tensor_copy(attnT[:, j * SB : (j + 1) * SB], t2[:])

            # softmax over s (free axis) per batch
            mx = smv_pool.tile([128, 1], F32)
            nc.vector.reduce_max(out=mx[:], in_=attnT[:], axis=mybir.AxisListType.X)
            negmx = smv_pool.tile([128, 1], F32)
            nc.vector.tensor_scalar_mul(negmx[:], mx[:], -1.0)
            p_sb = sm_pool.tile([128, S], F32)
            ssum = smv_pool.tile([128, 1], F32)
            nc.scalar.activation(
                p_sb[:], attnT[:], AF.Exp,
                bias=negmx[:, 0:1], scale=1.0, accum_out=ssum[:, 0:1],
            )
            rcp = smv_pool.tile([128, 1], F32)
            nc.vector.reciprocal(rcp[:], ssum[:])
            nc.vector.tensor_scalar_mul(p_sb[:], p_sb[:], rcp[:, 0:1])

            # scaled half of the output: in-place scale of xg (out0 for the
            # block has already been stored), then line-rate store
            for j in range(4):
                s0 = j * SB
                pslice = p_sb[:, s0 : s0 + SB]
                pb = bass.AP(
                    tensor=pslice.tensor,
                    offset=pslice.offset,
                    ap=list(pslice.ap) + [[0, E]],
                )
                nc.vector.tensor_tensor(
                    out=xg[h][:, s0 : s0 + SB, :], in0=xg[h][:, s0 : s0 + SB, :],
                    in1=pb, op=mybir.AluOpType.mult,
                )
                nc.sync.dma_start(
                    out=out_d[b0 : b0 + 128, 1, s0 : s0 + SB, :],
                    in_=xg[h][:, s0 : s0 + SB, :],
                )
    _split_multi_waits(nc)
    return nc


_NC_CACHE = None


def _get_program():
    global _NC_CACHE
    if _NC_CACHE is None:
        _NC_CACHE = _build_program()
    return _NC_CACHE


def kernel(input, W, bias, proj, _want_trace=False, _trace_dir=None):
    import ml_dtypes

    x = np.ascontiguousarray(np.asarray(input, dtype=np.float32))
    W = np.asarray(W, dtype=np.float32)
    bias = np.asarray(bias, dtype=np.float32)
    proj = np.asarray(proj, dtype=np.float32)
    assert x.shape == (S, B, E)

    w4 = np.zeros((KB, KB), np.float32)
    b4 = np.zeros((KB, 1), np.float32)
    p4 = np.zeros((KB, 256), np.float32)
    for g in range(4):
        w4[g * E : (g + 1) * E, g * E : (g + 1) * E] = W
        b4[g * E : (g + 1) * E, 0] = bias[:, 0]
        # slot matmul writes attn partitions 4*slot+g from contraction block g
        for slot in range(8):
            p4[g * E : (g + 1) * E, 32 * slot + 4 * slot + g] = proj[:, 0]
    w4_16 = w4.astype(ml_dtypes.bfloat16)
    p4_16 = p4.astype(ml_dtypes.bfloat16)

    nc = _get_program()
    in_maps = []
    for c in range(N_CORES):
        shard = np.ascontiguousarray(x[:, c * BC : (c + 1) * BC, :])
        in_maps.append(
            {"input": shard, "W4": w4_16, "bias4": b4, "proj4": p4_16}
        )

    res = run_bass_kernel_spmd(
        nc, in_maps, list(range(N_CORES)), trace=_want_trace, tmpdir=_trace_dir
    )
    out = np.concatenate([res.results[c]["output"] for c in range(N_CORES)], axis=0)
    if _want_trace:
        return out, res
    return out


# revision 19
# speedup vs baseline: 1.5127x; 1.2671x over previous
"""Trainium2 Bass kernel for nn_AttentionToken.

reference semantics (per full input (S=512, B=2048, E=30)):
    squish  = tanh(x @ W + bias[:,0])          # (S,B,E)
    attn    = tanh(squish @ proj[:,0])         # (S,B)
    attn_n  = softmax over S, per batch        # (B,S)
    out     = stack([x.T(b,s,e), x.T * attn_n[:, :, None]], axis=1)  # (B,2,S,E)

Sharding: data-parallel over batch, 8 cores x 256 batches.

Per-core design (v3, fp32r pipeline, no conversion stage):
  - x loaded s-major (s on partitions, (b,e) free) at DMA line rate.
  - b-major fp32 copy (xg) built with fp32r PE transposes (1.5 cycles/row)
    whose PSUM results drain in batched copies alternating DVE/Scalar;
    out0 stores stream from xg at line rate on the SP queue.
  - matmul path: 128x120 fp32r PE transposes pack 4 batches x 30 embed onto
    120 partitions; the 30x30 GEMM runs as a block-diagonal 120x120 fp32r
    matmul with a 512-wide moving operand (1 cycle/row).  tanh on Scalar.
  - proj contraction keeps a mostly-zero (120, 32) proj matrix stationary per
    slot; 8 slot matmuls accumulate one (32, 512) attention PSUM tile whose
    (partition, col-group) layout maps affinely to (batch, s).
  - two small PE transpose rounds turn attention into natural (b, s) layout;
    softmax over the free axis; in-place scale of xg; out1 stores dispatched
    from the DVE queue so they chase the scale ops without head-blocking
    the SP load/store queue.
  - PE-side software pipelining: transposes are emitted before the dependent
    matmuls of the same block so the in-order PE queue never waits on
    DVE/Scalar round trips.
"""

from contextlib import ExitStack

import numpy as np

import concourse.bass as bass
import concourse.tile as tile
from concourse import mybir
from concourse.bass_utils import run_bass_kernel_spmd
from concourse.masks import make_identity
from concourse.vector_clock import ScopedClock


class _TileContextSplitDrain(tile.TileContext):
    """TileContext whose exit drain stays within the 1-sem-wait-per-
    instruction encoding limit of this walrus build.

    The stock ``_drain_and_barrier`` attaches the whole global clock to a
    single Drain, which codegen rejects ("Too many sync wait commands").
    Emit one standalone SP wait per semaphore instead, then a clean drain.
    """

    def _drain_and_barrier(self, tick_clock, wait_clock):
        nc = self.nc
        with nc.discard():
            probe = nc.sync.drain()
            wait_clock.add_sem_waits(
                probe.ins, ScopedClock({None: tick_clock.global_clock})
            )
            si = probe.ins.sync_info
            waits = list(si.on_wait) if si and si.on_wait else []
        assert self.sems is not None
        alloc = self.sems.allocated()
        by_num = {h.num: h for h in alloc.values()}
        for w in waits:
            h = by_num.get(w.id)
            assert h is not None, (w.id, w.ant_name, sorted(by_num))
            nc.sync.wait_ge(h, w.wait_value)
        nc.sync.drain()
        nc.all_engine_barrier()
        popped = nc._tile_sem_poison_stack.pop()
        assert popped is self._sem_poison
        nc.clear_and_free_semaphores(list(alloc.values()))
        nc.all_engine_barrier()

S = 512
B = 2048
E = 30
N_CORES = 8
BC = B // N_CORES          # batches per core (256)
SB = 128                   # s-block size (s on partitions)
KB = 4 * E                 # 120: block-diag contraction/output size
F32 = mybir.dt.float32
F32R = mybir.dt.float32r
BF16 = mybir.dt.bfloat16
AF = mybir.ActivationFunctionType


def _split_multi_waits(nc, max_waits=1):
    """This walrus build encodes at most one sem-wait per instruction; the
    Tile scheduler emits up to ~3.  Hoist extra waits onto standalone
    EventSemaphore instructions on the same engine, just before the owner.
    """
    n = 0
    for f in nc.m.functions:
        for bb in f.blocks:
            out = []
            for ins in bb.instructions:
                si = ins.sync_info
                waits = list(si.on_wait) if si and si.on_wait else []
                if len(waits) > max_waits:
                    for w in waits[:-max_waits]:
                        ev = mybir.InstEventSemaphore(
                            name=f"wsplit-{n}",
                            opcode="EventSemaphore",
                            engine=ins.engine,
                            sync_info=mybir.SyncInfo(on_wait=[w], on_update=[]),
                        )
                        n += 1
                        out.append(ev)
                    ins.sync_info = mybir.SyncInfo(
                        on_wait=waits[-max_waits:],
                        on_update=list(si.on_update or []),
                    )
                out.append(ins)
            bb.instructions = out


def _reorder_free(ap, perm):
    """Return the same AP with its free dims permuted (partition dim kept)."""
    dims = list(ap.ap)
    free = dims[1:]
    return bass.AP(
        tensor=ap.tensor,
        offset=ap.offset,
        ap=[dims[0]] + [free[i] for i in perm],
    )


def _r(ap):
    return ap.bitcast(F32R)


def _build_program():
    nc = bass.Bass()
    x_d = nc.declare_dram_parameter("input", [S, BC, E], F32R, isOutput=False)
    w4_d = nc.declare_dram_parameter("W4", [KB, KB], BF16, isOutput=False)
    b4_d = nc.declare_dram_parameter("bias4", [KB, 1], F32, isOutput=False)
    p4_d = nc.declare_dram_parameter("proj4", [KB, 256], BF16, isOutput=False)
    id_d = nc.declare_dram_parameter("ident", [128, 128], F32R, isOutput=False)
    out_d = nc.declare_dram_parameter("output", [BC, 2, S, E], F32, isOutput=True)

    # e-slice transpose batches: (start, count) draining into one PSUM tile
    EBATCH = [(0, 4), (4, 4), (8, 4), (12, 4), (16, 4), (20, 4), (24, 4), (28, 2)]

    with _TileContextSplitDrain(nc) as tc, ExitStack() as ctx:
        consts = ctx.enter_context(tc.tile_pool(name="consts", bufs=1))
        xs_pool = ctx.enter_context(tc.tile_pool(name="xs", bufs=3))
        xg_pool = ctx.enter_context(tc.tile_pool(name="xg", bufs=1))
        xt_pool = ctx.enter_context(tc.tile_pool(name="xt", bufs=3))
        sq_pool = ctx.enter_context(tc.tile_pool(name="sq", bufs=4))
        at_pool = ctx.enter_context(tc.tile_pool(name="at", bufs=6))
        atS_pool = ctx.enter_context(tc.tile_pool(name="atS", bufs=4))
        atT_pool = ctx.enter_context(tc.tile_pool(name="atT", bufs=2))
        sm_pool = ctx.enter_context(tc.tile_pool(name="sm", bufs=2))
        smv_pool = ctx.enter_context(tc.tile_pool(name="smv", bufs=8))
        ps_tp = ctx.enter_context(tc.tile_pool(name="ps_tp", bufs=2, space="PSUM"))
        ps_xt = ctx.enter_context(tc.tile_pool(name="ps_xt", bufs=2, space="PSUM"))
        ps_sq = ctx.enter_context(tc.tile_pool(name="ps_sq", bufs=2, space="PSUM"))
        ps_sm = ctx.enter_context(tc.tile_pool(name="ps_sm", bufs=2, space="PSUM"))

        identf = consts.tile([128, 128], F32)
        make_identity(nc, identf[:])
        identr = consts.tile([128, 128], F32R)
        nc.sync.dma_start(out=identr[:], in_=id_d[:, :])
        w4_sb = consts.tile([KB, KB], BF16)
        nc.sync.dma_start(out=w4_sb[:], in_=w4_d[:, :])
        b4_sb = consts.tile([KB, 1], F32)
        nc.sync.dma_start(out=b4_sb[:], in_=b4_d[:, :])
        p4_sb = consts.tile([KB, 256], BF16)
        nc.sync.dma_start(out=p4_sb[:], in_=p4_d[:, :])

        xg = [
            xg_pool.tile([128, S, E], F32, name=f"xg{i}", tag=f"xg{i}")
            for i in range(2)
        ]

        for h in range(2):
            b0 = h * 128
            attn_sbs = []
            for j in range(4):
                s0 = j * SB
                # line-rate load: s on partitions, (b, e) contiguous 15.4KB/run
                xs = xs_pool.tile([SB, 128, E], F32R)
                nc.sync.dma_start(
                    out=xs[:], in_=x_d[s0 : s0 + SB, b0 : b0 + 128, :]
                )

                # b-major conversion: per-e fp32r PE transposes (s,b)->(b,s),
                # drained in batched copies alternating DVE / Scalar
                for bi, (e0, ne) in enumerate(EBATCH):
                    tp = ps_tp.tile([128, 512], F32)
                    for k in range(ne):
                        nc.tensor.transpose(
                            _r(tp[:, k * 128 : (k + 1) * 128]),
                            xs[:, :, e0 + k],
                            identr[:],
                        )
                    sl = xg[h][:, s0 : s0 + SB, e0 : e0 + ne]
                    dst = _reorder_free(sl, [1, 0])  # (e outer, s inner)
                    if bi % 2 == 0:
                        nc.vector.tensor_copy(dst, tp[:, : ne * 128])
                    else:
                        nc.scalar.copy(dst, tp[:, : ne * 128])
                # unscaled half of the output: independent of attn
                nc.sync.dma_start(
                    out=out_d[b0 : b0 + 128, 0, s0 : s0 + SB, :],
                    in_=xg[h][:, s0 : s0 + SB, :],
                )

                # matmul path: quad (slot, c) holds batches b0 + 32c + 4*slot + i
                # so attention lands at (partition 4*slot+i, col-group c) with
                # batch-in-half = 32c + partition.  All transposes first, then
                # the W/proj matmuls one slot behind, so the in-order PE queue
                # never waits on the DVE/Scalar round trips.
                attn_ps = ps_sm.tile([32, 512], F32, tag="atn")
                xt_sbs = []
                for slot in range(8):
                    xt_ps = ps_xt.tile([KB, 512], F32)
                    for c in range(4):
                        boff = 32 * c + 4 * slot
                        nc.tensor.transpose(
                            _r(xt_ps[:, c * 128 : (c + 1) * 128]),
                            xs[:, boff : boff + 4, :],
                            identr[:],
                        )
                    xt_sb = xt_pool.tile([KB, 512], BF16)
                    nc.vector.tensor_copy(xt_sb[:], xt_ps[:])
                    xt_sbs.append(xt_sb)

                sq_sbs = []
                for slot in range(8):
                    sq_ps = ps_sq.tile([KB, 512], F32)
                    nc.tensor.matmul(
                        sq_ps[:], w4_sb[:], xt_sbs[slot][:],
                        start=True, stop=True,
                    )
                    sq_sb = sq_pool.tile([KB, 512], BF16)
                    nc.scalar.activation(
                        sq_sb[:], sq_ps[:], AF.Tanh,
                        bias=b4_sb[:, 0:1], scale=1.0,
                    )
                    sq_sbs.append(sq_sb)
                    if slot >= 1:
                        # proj matmul for the previous slot: its tanh ran
                        # while this slot's W matmul was on the PE
                        ps = slot - 1
                        nc.tensor.matmul(
                            attn_ps[:, :],
                            p4_sb[:, 32 * ps : 32 * ps + 32],
                            sq_sbs[ps][:],
                            start=(ps == 0), stop=False,
                        )
                nc.tensor.matmul(
                    attn_ps[:, :], p4_sb[:, 224:256], sq_sbs[7][:],
                    start=False, stop=True,
                )
                attn_sb = at_pool.tile([32, 512], F32)
                nc.scalar.activation(attn_sb[:], attn_ps[:], AF.Tanh)
                attn_sbs.append(attn_sb)

            # bridge: (32 part, (c, s)) attn tiles -> natural (b, s) layout.
            # All first-round transposes, then the second round, so the PE
            # queue isn't blocked on the DVE copies in between.
            t1s = []
            for j in range(4):
                t1 = ps_sm.tile([128, 128], F32, tag="atn")
                for c in range(4):
                    nc.tensor.transpose(
                        t1[:, 32 * c : 32 * c + 32],
                        attn_sbs[j][:, 128 * c : 128 * c + 128],
                        identf[0:32, 0:32],
                    )
                attnS = atS_pool.tile([128, 128], F32)
                nc.vector.tensor_copy(attnS[:], t1[:])
                t1s.append(attnS)
            attnT = atT_pool.tile([128, S], F32)
            for j in range(4):
                t2 = ps_sm.tile([128, 128], F32, tag="atn")
                nc.tensor.transpose(t2[:], t1s[j][:], identf[:])
                nc.vector.tensor_copy(attnT[:, j * SB : (j + 1) * SB], t2[:])

            # softmax over s (free axis) per batch
            mx = smv_pool.tile([128, 1], F32)
            nc.vector.reduce_max(out=mx[:], in_=attnT[:], axis=mybir.AxisListType.X)
            negmx = smv_pool.tile([128, 1], F32)
            nc.vector.tensor_scalar_mul(negmx[:], mx[:], -1.0)
            p_sb = sm_pool.tile([128, S], F32)
            ssum = smv_pool.tile([128, 1], F32)
            nc.scalar.activation(
                p_sb[:], attnT[:], AF.Exp,
                bias=negmx[:, 0:1], scale=1.0, accum_out=ssum[:, 0:1],
            )
            rcp = smv_pool.tile([128, 1], F32)
            nc.vector.reciprocal(rcp[:], ssum[:])
            nc.vector.tensor_scalar_mul(p_sb[:], p_sb[:], rcp[:, 0:1])

            # scaled half of the output: in-place scale of xg (out0 for the
            # block has already been stored), then line-rate store chased on
            # the DVE queue right behind its scale op
            for j in range(4):
                s0 = j * SB
                pslice = p_sb[:, s0 : s0 + SB]
                pb = bass.AP(
                    tensor=pslice.tensor,
                    offset=pslice.offset,
                    ap=list(pslice.ap) + [[0, E]],
                )
                nc.vector.tensor_tensor(
                    out=xg[h][:, s0 : s0 + SB, :], in0=xg[h][:, s0 : s0 + SB, :],
                    in1=pb, op=mybir.AluOpType.mult,
                )
                nc.gpsimd.dma_start(
                    out=out_d[b0 : b0 + 128, 1, s0 : s0 + SB, :],
                    in_=xg[h][:, s0 : s0 + SB, :],
                )
    _split_multi_waits(nc)
    return nc


_NC_CACHE = None


def _get_program():
    global _NC_CACHE
    if _NC_CACHE is None:
        _NC_CACHE = _build_program()
    return _NC_CACHE


def kernel(input, W, bias, proj, _want_trace=False, _trace_dir=None):
    import ml_dtypes

    x = np.ascontiguousarray(np.asarray(input, dtype=np.float32))
    W = np.asarray(W, dtype=np.float32)
    bias = np.asarray(bias, dtype=np.float32)
    proj = np.asarray(proj, dtype=np.float32)
    assert x.shape == (S, B, E)

    w4 = np.zeros((KB, KB), np.float32)
    b4 = np.zeros((KB, 1), np.float32)
    p4 = np.zeros((KB, 256), np.float32)
    for g in range(4):
        w4[g * E : (g + 1) * E, g * E : (g + 1) * E] = W
        b4[g * E : (g + 1) * E, 0] = bias[:, 0]
        # slot matmul writes attn partitions 4*slot+g from contraction block g
        for slot in range(8):
            p4[g * E : (g + 1) * E, 32 * slot + 4 * slot + g] = proj[:, 0]

    w4_16 = w4.astype(ml_dtypes.bfloat16)
    p4_16 = p4.astype(ml_dtypes.bfloat16)
    ident = np.eye(128, dtype=np.float32)

    nc = _get_program()
    in_maps = []
    for c in range(N_CORES):
        shard = np.ascontiguousarray(x[:, c * BC : (c + 1) * BC, :])
        in_maps.append(
            {"input": shard, "W4": w4_16, "bias4": b4, "proj4": p4_16,
             "ident": ident}
        )

    res = run_bass_kernel_spmd(
        nc, in_maps, list(range(N_CORES)), trace=_want_trace, tmpdir=_trace_dir
    )
    out = np.concatenate([res.results[c]["output"] for c in range(N_CORES)], axis=0)
    if _want_trace:
        return out, res
    return out
